# revision 14
# baseline (speedup 1.0000x reference)
"""Trainium2 Bass kernel for nn_DKOKernel (dense pairwise MLP + PSD head).

Math (per batch b, handled by core b; B=8=n_cores):
  hx[f,i] = sum_c wx[f,c] x[b,i,c];  hy[f,j] = sum_c wy[f,c] y[b,j,c]
  h1 = relu(hx_i + hy_j)                    (512, bn1 folded)
  h2 = relu(W2' h1 + c2)                    (256)
  h3 = relu(W3' h2 + c3)                    (128)
  q_i  = sum_j h3_ij
  s_i  = W4 q_i + ny*b4
  v_i  = W4^T s_i ;  c_i = b4 . s_i
  out[i,j] = h3_ij . v_i + c_i

All activations and big matmuls in bf16 (PSUM accumulates fp32);
measured end-to-end error ~4e-3 vs the 2e-2 gate. Tiny head matmuls
(s, v, c) in plain fp32.

Loop: 32 chunks of C=4 i-rows (free dim 512). L1 = 16 [128,128]
broadcast-add-relu ops split DVE/ACT. h3 accumulates into a quad-wide
tile; q reduced once per 4 chunks; head (s/v/c + dot-product matmuls
with h3 as the stationary operand) batched over 4 chunks. out[j,i] is
assembled transposed in PSUM ([128,32] per 8 chunks), ACT-evicted, and
DMA'd once; the host transposes back.
"""

import os
import numpy as np
import ml_dtypes
from contextlib import ExitStack

import concourse.bacc as bacc
import concourse.tile as tile
from concourse import mybir
from concourse.bass_utils import run_bass_kernel_spmd

F32 = mybir.dt.float32
BF16 = mybir.dt.bfloat16
AF = mybir.ActivationFunctionType
ALU = mybir.AluOpType
AX = mybir.AxisListType

EPS = 1e-5
B = 8
N = 128
F = 128
D1, D2, D3, D4 = 512, 256, 128, 64
C = 4            # i-rows per chunk
NCH = N // C     # 32 chunks
HB = 4           # chunks per head batch
OG = 8           # chunks per output-psum group (OG*C = 32 columns)

# L1 engine assignment: 4 tokens (per fc), each 4 chars (per ii): V=DVE, A=ACT
L1_ASSIGN = os.environ.get("L1_ASSIGN", "VVVV,VVVV,VVAA,AVVA").split(",")
WORK_BUFS = int(os.environ.get("WORK_BUFS", "3"))


def build_module():
    nc = bacc.Bacc()

    xT = nc.declare_dram_parameter("xT", [F, N], BF16, isOutput=False)
    yT = nc.declare_dram_parameter("yT", [F, N], BF16, isOutput=False)
    wxT = nc.declare_dram_parameter("wxT", [F, D1], BF16, isOutput=False)
    wyT = nc.declare_dram_parameter("wyT", [F, D1], BF16, isOutput=False)
    w2T = nc.declare_dram_parameter("w2T", [4, 128, D2], BF16, isOutput=False)
    w3T = nc.declare_dram_parameter("w3T", [2, 128, D3], BF16, isOutput=False)
    w4T = nc.declare_dram_parameter("w4T", [128, D4], F32, isOutput=False)
    w4N = nc.declare_dram_parameter("w4N", [D4, 128], F32, isOutput=False)
    b4c = nc.declare_dram_parameter("b4c", [D4], F32, isOutput=False)
    c1d = nc.declare_dram_parameter("c1", [4, 128], F32, isOutput=False)
    c2d = nc.declare_dram_parameter("c2", [2, 128], F32, isOutput=False)
    c3d = nc.declare_dram_parameter("c3", [128], F32, isOutput=False)
    b4x = nc.declare_dram_parameter("b4x", [D4], F32, isOutput=False)
    out_d = nc.declare_dram_parameter("outT", [N, N], F32, isOutput=True)

    with tile.TileContext(nc) as tc:
        with ExitStack() as ctx:
            singles = ctx.enter_context(tc.tile_pool(name="singles", bufs=1))

            xT_s = singles.tile([F, N], BF16)
            yT_s = singles.tile([F, N], BF16)
            wxT_s = singles.tile([F, D1], BF16)
            wyT_s = singles.tile([F, D1], BF16)
            w2_s = singles.tile([128, 4, D2], BF16)
            w3_s = singles.tile([128, 2, D3], BF16)
            w4T_s = singles.tile([128, D4], F32)
            w4N_s = singles.tile([D4, 128], F32)
            b4c_s = singles.tile([D4, 1], F32)
            c1_s = singles.tile([128, 4], F32)
            c2_s = singles.tile([128, 2], F32)
            c3_s = singles.tile([128, 1], F32)
            b4x_s = singles.tile([D4, 1], F32)
            ones_row = singles.tile([1, 128], BF16)
            hx_s = singles.tile([128, 4, N], F32)
            hy_s = singles.tile([128, 4, N], BF16)
            out_sT = singles.tile([N, N], F32)

            nc.sync.dma_start(out=xT_s, in_=xT[:, :])
            nc.sync.dma_start(out=yT_s, in_=yT[:, :])
            nc.sync.dma_start(out=wxT_s, in_=wxT[:, :])
            nc.sync.dma_start(out=wyT_s, in_=wyT[:, :])
            for kc in range(4):
                nc.sync.dma_start(out=w2_s[:, kc, :], in_=w2T[kc, :, :])
            for kc in range(2):
                nc.sync.dma_start(out=w3_s[:, kc, :], in_=w3T[kc, :, :])
            nc.sync.dma_start(out=w4T_s, in_=w4T[:, :])
            nc.sync.dma_start(out=w4N_s, in_=w4N[:, :])
            nc.sync.dma_start(out=b4c_s[:, 0], in_=b4c[:])
            for fc in range(4):
                nc.sync.dma_start(out=c1_s[:, fc], in_=c1d[fc, :])
            for mc in range(2):
                nc.sync.dma_start(out=c2_s[:, mc], in_=c2d[mc, :])
            nc.sync.dma_start(out=c3_s[:, 0], in_=c3d[:])
            nc.sync.dma_start(out=b4x_s[:, 0], in_=b4x[:])
            nc.vector.memset(ones_row, 1.0)

            with tc.tile_pool(name="psum_setup", bufs=2, space="PSUM") as pp:
                for fc in range(4):
                    ph0 = pp.tile([128, N], F32, tag="ph")
                    nc.tensor.matmul(
                        ph0, lhsT=wxT_s[:, fc * 128:(fc + 1) * 128],
                        rhs=xT_s, start=True, stop=True)
                    nc.scalar.activation(hx_s[:, fc, :], ph0, AF.Copy)
                    py_ = pp.tile([128, N], F32, tag="ph")
                    nc.tensor.matmul(
                        py_, lhsT=wyT_s[:, fc * 128:(fc + 1) * 128],
                        rhs=yT_s, start=True, stop=True)
                    nc.scalar.activation(hy_s[:, fc, :], py_, AF.Identity,
                                         bias=c1_s[:, fc:fc + 1])

            work = ctx.enter_context(tc.tile_pool(name="work",
                                                  bufs=WORK_BUFS))
            hpool = ctx.enter_context(tc.tile_pool(name="hpool", bufs=2))
            ps2 = ctx.enter_context(tc.tile_pool(name="ps2", bufs=2,
                                                 space="PSUM"))
            ps3 = ctx.enter_context(tc.tile_pool(name="ps3", bufs=2,
                                                 space="PSUM"))
            psS = ctx.enter_context(tc.tile_pool(name="psS", bufs=2,
                                                 space="PSUM"))
            psO = ctx.enter_context(tc.tile_pool(name="psO", bufs=2,
                                                 space="PSUM"))

            out_ps = None
            h34 = None
            for t in range(NCH):
                tb = t % HB
                # ---- L1 ----
                h1 = work.tile([128, 4, C * N], BF16, tag="h1")
                for fc in range(4):
                    mode = L1_ASSIGN[fc]
                    for ii in range(C):
                        sl = slice(ii * N, (ii + 1) * N)
                        xc = hx_s[:, fc, C * t + ii:C * t + ii + 1]
                        if mode[ii] == "V":
                            nc.vector.tensor_scalar(
                                out=h1[:, fc, sl],
                                in0=hy_s[:, fc, :],
                                scalar1=xc, scalar2=0.0,
                                op0=ALU.add, op1=ALU.max)
                        else:
                            nc.scalar.activation(
                                h1[:, fc, sl], hy_s[:, fc, :],
                                AF.Relu, bias=xc)

                # ---- L2 ----
                h2 = work.tile([128, 2, C * N], BF16, tag="h2")
                for mc in range(2):
                    p2 = ps2.tile([128, C * N], F32, tag="p2")
                    for kc in range(4):
                        nc.tensor.matmul(
                            p2,
                            lhsT=w2_s[:, kc, mc * 128:(mc + 1) * 128],
                            rhs=h1[:, kc, :],
                            start=(kc == 0), stop=(kc == 3))
                    nc.scalar.activation(h2[:, mc, :], p2, AF.Relu,
                                         bias=c2_s[:, mc:mc + 1])

                # ---- L3 ----
                p3 = ps3.tile([128, C * N], F32, tag="p3")
                for kc in range(2):
                    nc.tensor.matmul(
                        p3, lhsT=w3_s[:, kc, :], rhs=h2[:, kc, :],
                        start=(kc == 0), stop=(kc == 1))
                if tb == 0:
                    h34 = hpool.tile([128, HB, C * N], BF16, tag="h34")
                nc.vector.tensor_scalar(
                    out=h34[:, tb, :], in0=p3,
                    scalar1=c3_s, scalar2=0.0,
                    op0=ALU.add, op1=ALU.max)

                if t % OG == 0:
                    out_ps = psO.tile([128, OG * C], F32, tag="po")

                # ---- head (batched over HB chunks) ----
                if tb == HB - 1:
                    CH = HB * C
                    q4 = work.tile([128, CH], F32, tag="q")
                    nc.vector.tensor_reduce(
                        out=q4,
                        in_=h34.rearrange("p h (a b) -> p (h a) b", a=C),
                        axis=AX.X, op=ALU.add)

                    phd = psS.tile([128, 3 * CH], F32, tag="sh")
                    nc.tensor.matmul(phd[0:D4, 0:CH], lhsT=w4T_s, rhs=q4,
                                     start=True, stop=True)
                    s_sb = work.tile([D4, CH], F32, tag="s")
                    nc.scalar.activation(s_sb, phd[0:D4, 0:CH], AF.Identity,
                                         bias=b4x_s)

                    nc.tensor.matmul(phd[:, CH:2 * CH], lhsT=w4N_s,
                                     rhs=s_sb, start=True, stop=True)
                    v_sb = work.tile([128, CH], BF16, tag="v")
                    nc.scalar.activation(v_sb, phd[:, CH:2 * CH], AF.Copy)

                    nc.tensor.matmul(phd[0:1, 2 * CH:3 * CH], lhsT=b4c_s,
                                     rhs=s_sb, start=True, stop=True)
                    c_sb = work.tile([1, CH], BF16, tag="c")
                    nc.scalar.activation(c_sb, phd[0:1, 2 * CH:3 * CH],
                                         AF.Copy)

                    cb = ((t - (HB - 1)) % OG) * C
                    nc.tensor.matmul(
                        out_ps[:, cb:cb + CH],
                        lhsT=ones_row,
                        rhs=c_sb[0:1, :],
                        start=True, stop=False)
                    for k in range(CH):
                        ii = k % C
                        nc.tensor.matmul(
                            out_ps[:, cb + k:cb + k + 1],
                            lhsT=h34[:, k // C, ii * N:(ii + 1) * N],
                            rhs=v_sb[:, k:k + 1],
                            start=False, stop=True)

                if t % OG == OG - 1:
                    g = t // OG
                    nc.scalar.activation(
                        out_sT[:, g * OG * C:(g + 1) * OG * C],
                        out_ps, AF.Copy)

            nc.sync.dma_start(out=out_d[:, :], in_=out_sT)
    nc.finalize()
    return nc


_NC_CACHE = None


def _get_nc():
    global _NC_CACHE
    if _NC_CACHE is None:
        _NC_CACHE = build_module()
    return _NC_CACHE


def host_prep(inputs):
    """Fold BatchNorm affines into weights/biases; pre-transpose into the
    device layouts (bf16 for the big operands)."""
    f32 = np.float32
    bf = ml_dtypes.bfloat16
    x = np.asarray(inputs["x"], f32)
    y = np.asarray(inputs["y"], f32)
    w1 = np.asarray(inputs["w1"], f32)
    b1 = np.asarray(inputs["b1"], f32)
    w2, b2 = np.asarray(inputs["w2"], f32), np.asarray(inputs["b2"], f32)
    w3, b3 = np.asarray(inputs["w3"], f32), np.asarray(inputs["b3"], f32)
    w4, b4 = np.asarray(inputs["w4"], f32), np.asarray(inputs["b4"], f32)

    k1 = inputs["g1"] / np.sqrt(inputs["v1"] + EPS)
    c1 = k1 * (b1 - inputs["m1"]) + inputs["be1"]
    k2 = inputs["g2"] / np.sqrt(inputs["v2"] + EPS)
    c2 = k2 * (b2 - inputs["m2"]) + inputs["be2"]
    k3 = inputs["g3"] / np.sqrt(inputs["v3"] + EPS)
    c3 = k3 * (b3 - inputs["m3"]) + inputs["be3"]

    wx = w1[:, :F] * k1[:, None]
    wy = w1[:, F:] * k1[:, None]
    w2f = w2 * k2[:, None]
    w3f = w3 * k3[:, None]

    shared = {
        "wxT": np.ascontiguousarray(wx.T).astype(bf),
        "wyT": np.ascontiguousarray(wy.T).astype(bf),
        "w2T": np.ascontiguousarray(w2f.T.reshape(4, 128, D2)).astype(bf),
        "w3T": np.ascontiguousarray(w3f.T.reshape(2, 128, D3)).astype(bf),
        "w4T": np.ascontiguousarray(w4.T, f32),
        "w4N": np.ascontiguousarray(w4, f32),
        "b4c": np.ascontiguousarray(b4, f32),
        "c1": np.ascontiguousarray(c1.reshape(4, 128), f32),
        "c2": np.ascontiguousarray(c2.reshape(2, 128), f32),
        "c3": np.ascontiguousarray(c3, f32),
        "b4x": np.ascontiguousarray(N * b4, f32),
    }
    in_maps = []
    for b in range(B):
        m = dict(shared)
        m["xT"] = np.ascontiguousarray(x[b].T).astype(bf)
        m["yT"] = np.ascontiguousarray(y[b].T).astype(bf)
        in_maps.append(m)
    return in_maps


def kernel(**inputs):
    nc = _get_nc()
    in_maps = host_prep(inputs)
    res = run_bass_kernel_spmd(nc, in_maps, list(range(B)))
    out = np.stack([res.results[b]["outT"].T for b in range(B)], axis=0)
    return np.ascontiguousarray(out).astype(np.float32)
